# revision 37
# baseline (speedup 1.0000x reference)
"""Multi-head attention (RoPE) Trainium2 Bass kernel — v4.

Problem: B=4, T=2048, C=1024, H=16, d=64, fp32 in/out, full attention + RoPE.
Sharding: 8 cores = 4 batches x 2 head-groups (8 heads each). Each core
computes its batch's attention for its heads plus the partial output
projection; the host sums the two head-group partials per batch.

v4 design (ACT exp stream is the binding resource at ~266us):
- exp on ACT with FD=1024 over a 2+2 PSUM bank rotation (s0/s1)
- scores run in fp8e4m3 + DoubleRow (0.5 cyc/row): q/k are stored in a
  (d, d+32)-paired fp8 layout [128p = 4 heads x 32, dd in {0,1}, T] so the
  d-halves of each head share partitions. This halves score PE time
  (109us -> 55us) and makes rotate_half a free-dim swap (no PE permute
  matmul): rope runs as mul/mul/sub-add chains on DVE + GPSIMD.
  q/k carry a 16x scale (weights 16x in fp8, rope tables 16x) so fp8
  stays out of the subnormal range; exp rescales by 1/256.
- the q/k projection runs in fp8 + DoubleRow over c-chunk pairs as before
- v and the exp outputs stay bf16 for precision (fp8 there puts ~4-6%
  relative error straight onto the output; fp8 on q/k pre-softmax only
  perturbs scores by ~0.9% rms which the softmax tolerates)
- AV flipped: out [q x 65] with exp slices stationary; a ones-column
  yields softmax denominators per-query-partition
- attn output transposed to feature-major via PE transpose for the
  output projection
- QKV/v/proj run as PE gap-fillers inside the ACT-bound attention loop
"""

import numpy as np

B, T, C = 4, 2048, 1024
H, D = 16, 64
G = 2                # head groups (cores per batch)
HG = H // G          # heads per core = 8
CC = C // 128        # 8 contraction chunks
NTB = T // 512       # 4 t-blocks
NKC = T // 128       # 16 key chunks
NJ = T // 128        # 16 query blocks
ROPE_BASE = 10000.0
SCALE = 1.0 / np.sqrt(D)
QK_SCALE = 16.0      # fp8 headroom scale carried by q and k

# wqk8 column-block order: (qk, f', dd) -> block index. k f'=0 first
# (the whole first j-pair's hc0/hc1 need it), then q f'=0, then f'=1.
BC = {("k", 0, 0): 0, ("k", 0, 1): 1, ("q", 0, 0): 2, ("q", 0, 1): 3,
      ("k", 1, 0): 4, ("k", 1, 1): 5, ("q", 1, 0): 6, ("q", 1, 1): 7}

_CACHED = {}


def _rope_tables():
    """cos/sin in the packed layout: row 32g+p of a 128-row tile carries
    frequency index p (same for every head g); 16x fp8-headroom scale."""
    inv_freq = 1.0 / (ROPE_BASE ** (np.arange(0, D, 2, dtype=np.float64) / D))
    t = np.arange(T, dtype=np.float64)
    freqs = np.outer(t, inv_freq)                 # (T, 32)
    cos = np.cos(freqs).T.astype(np.float32)      # (32, T)
    sin = np.sin(freqs).T.astype(np.float32)
    cosP = QK_SCALE * np.tile(cos, (4, 1))        # (128, T)
    sinP = QK_SCALE * np.tile(sin, (4, 1))
    return np.ascontiguousarray(cosP), np.ascontiguousarray(sinP)


def _attn_body(tc, outs, ins):
    import contextlib
    import concourse.mybir as mybir

    nc = tc.nc
    F32 = mybir.dt.float32
    BF16 = mybir.dt.bfloat16
    FP8 = mybir.dt.float8e4
    EXP = mybir.ActivationFunctionType.Exp
    IDENT = mybir.ActivationFunctionType.Identity
    DR = mybir.MatmulPerfMode.DoubleRow

    xT = ins["xT"]            # (1024, 2048) bf16 (for v)
    xT8 = ins["xT8"]          # (128, 16384) fp8, matches xt8_all layout
    wqk8 = ins["wqk8"]        # (128, 8192) fp8, matches wqk8_all, 16x scaled
    wv = ins["wv"]            # (1024, 512) bf16
    wproj = ins["wproj"]      # (512, 1024) bf16
    bqk = ins["bqk"]          # (128, 8) f32 per-block per-partition bias
    bv = ins["bv"]            # (128, 520) bf16 broadcast v bias + ones
    bproj = ins["bproj"]      # (128, 1024) bf16 broadcast proj bias
    csT_d = ins["csT"]        # (128, 4096) bf16 packed-rope [cos|sin] * 16
    out = outs["out"]         # (2048, 1024) f32 partial output

    ctx = contextlib.ExitStack()
    with ctx:
        pers = ctx.enter_context(tc.tile_pool(name="pers", bufs=1))

        # ---- persistent SBUF tiles ----
        cs_t = pers.tile([128, 2 * T], BF16, name="cs_t", tag="cs_t")
        cos_t = cs_t[:, 0:T]
        sin_t = cs_t[:, T:2 * T]
        bqk_t = pers.tile([128, 8], F32, name="bqk_t", tag="bqk_t")
        bv_t = pers.tile([128, 520], BF16, name="bv_t", tag="bv_t")
        bproj_t = pers.tile([128, 1024], BF16, name="bproj_t", tag="bproj_t")
        wv_all = pers.tile([128, 8 * 512], BF16, name="wv_all", tag="wv_all")
        wv_t = [wv_all[:, c * 512:(c + 1) * 512] for c in range(CC)]
        wp_all = pers.tile([128, 4 * 1024], BF16, name="wp_all", tag="wp_all")
        wproj_t = [wp_all[:, m * 1024:(m + 1) * 1024] for m in range(4)]
        # q/k in packed dd-pair fp8 layout, per (f', t-block):
        # [128 = 4 heads x 32 freq-rows, dd*512 + t]
        qk_q = [[pers.tile([128, 1024], FP8, name=f"q{f}_{tb}", tag=f"q{f}_{tb}")
                 for tb in range(NTB)] for f in range(2)]
        qk_k = [[pers.tile([128, 1024], FP8, name=f"k{f}_{tb}", tag=f"k{f}_{tb}")
                 for tb in range(NTB)] for f in range(2)]
        # fp8 DoubleRow inputs for the q/k projection: c-chunk pairs.
        # One wide tile each so the whole load is a single 4D-AP DMA.
        xt8_all = pers.tile([128, 4 * 4096], FP8, name="xt8_all", tag="xt8_all")
        xt8 = [xt8_all[:, p * 4096:(p + 1) * 4096] for p in range(4)]
        wqk8_all = pers.tile([128, 4 * 2048], FP8, name="wqk8_all",
                             tag="wqk8_all")
        wqk8_t = [wqk8_all[:, p * 2048:(p + 1) * 2048] for p in range(4)]
        # token-major v (+ones col per head)
        vg = [pers.tile([128, 520], BF16, name=f"vg{t}", tag=f"vg{t}")
              for t in range(NKC)]
        # normalized attn output, feature-major: per q-block j one
        # [128, 4x128] tile; block m = transpose of head-pair m's columns.
        # Rotating pool: aT(j) is dead once proj(j) has run, one jp later.
        aTp = ctx.enter_context(tc.tile_pool(name="aTp", bufs=4))
        aT_of = {}

        # ---- DMA loads, in prologue-need order ----
        nc.sync.dma_start(bqk_t, bqk)
        # critical quarters first, each as ONE multi-range strided DMA:
        # f'0 weight blocks, tb0/tb1 activations, first rope-table half
        w4v = wqk8_all.rearrange("r (P o t) -> r P o t", P=4, o=2)
        x4v = xt8_all.rearrange("r (P o t) -> r P o t", P=4, o=2)
        wq4 = wqk8.rearrange("r (P o t) -> r P o t", P=4, o=2)
        xT84 = xT8.rearrange("r (P o t) -> r P o t", P=4, o=2)
        cs3 = cs_t.rearrange("q (o t) -> q o t", o=2)
        csd3 = csT_d.rearrange("q (o t) -> q o t", o=2)
        nc.sync.dma_start(w4v[:, :, :, 0:512], wq4[:, :, :, 0:512])
        nc.sync.dma_start(x4v[:, :, :, 0:1024], xT84[:, :, :, 0:1024])
        nc.sync.dma_start(cs3[:, :, 0:1024], csd3[:, :, 0:1024])
        nc.sync.dma_start(x4v[:, :, :, 1024:2048], xT84[:, :, :, 1024:2048])
        nc.sync.dma_start(w4v[:, :, :, 512:1024], wq4[:, :, :, 512:1024])
        nc.sync.dma_start(cs3[:, :, 1024:2048], csd3[:, :, 1024:2048])
        nc.sync.dma_start(wv_all.rearrange("p (c t) -> p c t", c=8),
                          wv.rearrange("(c p) t -> p c t", c=8))
        nc.sync.dma_start(bv_t, bv)
        nc.sync.dma_start(wp_all.rearrange("p (m t) -> p m t", m=4),
                          wproj.rearrange("(m p) t -> p m t", m=4))
        nc.sync.dma_start(bproj_t, bproj)

        # ---- scratch pools ----
        expp = ctx.enter_context(tc.tile_pool(name="expp", bufs=41))
        xtp = ctx.enter_context(tc.tile_pool(name="xtp", bufs=2))
        xtv = {}

        def load_xt_tb(tb):
            big = xtp.tile([128, 8 * 512], BF16, name=f"xtv_{tb}", tag="xtv")
            nc.sync.dma_start(
                big.rearrange("p (c t) -> p c t", c=8),
                xT.rearrange("(c p) t -> p c t", c=8)[:, :,
                                                      tb * 512:(tb + 1) * 512])
            for c in range(CC):
                xtv[(c, tb)] = big[:, c * 512:(c + 1) * 512]
        rawp = ctx.enter_context(tc.tile_pool(name="rawp", bufs=2))
        nrmp = ctx.enter_context(tc.tile_pool(name="nrmp", bufs=2))
        rcpp = ctx.enter_context(tc.tile_pool(name="rcpp", bufs=2))
        outp = ctx.enter_context(tc.tile_pool(name="outp", bufs=2))
        # PSUM: s0(2) + s1(2) + av(2x1) + f(2x1) = 8 banks
        psA = ctx.enter_context(tc.tile_pool(name="psA", bufs=1, space="PSUM"))
        psAV = ctx.enter_context(tc.tile_pool(name="psAV", bufs=2, space="PSUM"))
        psF = ctx.enter_context(tc.tile_pool(name="psF", bufs=1, space="PSUM"))
        psV = ctx.enter_context(tc.tile_pool(name="psV", bufs=1, space="PSUM"))

        uid = [0]

        def fresh(n):
            uid[0] += 1
            return f"{n}_{uid[0]}"

        # ---------------- unit emitters ----------------
        def qkv2(qk, fp, tb, ext_on_act, borrow=None, borrow2=None):
            """q/k projection + rope for one (q-or-k, f'-group, t-block):
            both dd halves. Emits 8 DR matmuls + 2 bias-extracts + the
            6-op rope chain; output lands in the packed fp8 tile."""
            tsl = slice(tb * 512, (tb + 1) * 512)
            if borrow is not None:
                P2 = [borrow[:, 0:512], borrow[:, 512:1024]]
            elif borrow2 is not None:
                P = borrow2.tile([128, 512], F32, name=fresh("pv"), tag="v")
                P2 = [P, P]
            else:
                P = psF.tile([128, 512], F32, name=fresh("pf"), tag="f")
                P2 = [P, P]
            raw = []
            for dd in range(2):
                bc = BC[(qk, fp, dd)]
                for p in range(4):
                    w3 = wqk8_t[p].rearrange("q (o t) -> q o t", o=2)
                    x3 = xt8[p].rearrange("q (o t) -> q o t", o=2)
                    nc.tensor.matmul(
                        P2[dd], w3[:, :, bc * 128:(bc + 1) * 128],
                        x3[:, :, tb * 512:(tb + 1) * 512],
                        start=(p == 0), stop=(p == 3), perf_mode=DR)
                r = rawp.tile([128, 512], BF16, name=fresh("raw"), tag=f"raw{dd}")
                if ext_on_act:
                    nc.scalar.activation(r, P2[dd], IDENT,
                                         bias=bqk_t[:, bc:bc + 1],
                                         scale=1.0 / 16.0)
                else:
                    nc.vector.tensor_scalar(r, P2[dd], 1.0 / 16.0,
                                            bqk_t[:, bc:bc + 1],
                                            mybir.AluOpType.mult,
                                            mybir.AluOpType.add)
                raw.append(r)
            # rope: out0 = raw0*cos - raw1*sin ; out1 = raw1*cos + raw0*sin
            tc0 = rawp.tile([128, 512], BF16, name=fresh("tc0"), tag="tc0")
            nc.vector.tensor_mul(tc0, raw[0], cos_t[:, tsl])
            ts0 = rawp.tile([128, 512], BF16, name=fresh("ts0"), tag="ts0")
            nc.gpsimd.tensor_tensor(ts0, raw[1], sin_t[:, tsl],
                                    mybir.AluOpType.mult)
            tc1 = rawp.tile([128, 512], BF16, name=fresh("tc1"), tag="tc1")
            nc.vector.tensor_mul(tc1, raw[1], cos_t[:, tsl])
            ts1 = rawp.tile([128, 512], BF16, name=fresh("ts1"), tag="ts1")
            nc.gpsimd.tensor_tensor(ts1, raw[0], sin_t[:, tsl],
                                    mybir.AluOpType.mult)
            dst = (qk_q if qk == "q" else qk_k)[fp][tb]
            nc.vector.tensor_tensor(dst[:, 0:512], tc0, ts0,
                                    mybir.AluOpType.subtract)
            nc.vector.tensor_tensor(dst[:, 512:1024], tc1, ts1,
                                    mybir.AluOpType.add)

        def qkv_v(t, P):
            tb, tr = t // 4, (t % 4) * 128
            for c in range(CC):
                nc.tensor.matmul(
                    P, xtv[(c, tb)][:, tr:tr + 128], wv_t[c],
                    start=(c == 0), stop=(c == CC - 1))
            vv = vg[t].rearrange("p (g d) -> p g d", g=HG)
            bvv = bv_t.rearrange("p (g d) -> p g d", g=HG)
            nc.vector.tensor_add(
                vv[:, :, 0:64], P.rearrange("p (g d) -> p g d", g=HG),
                bvv[:, :, 0:64])
            nc.vector.tensor_copy(vv[:, :, 64:65], bvv[:, :, 64:65])

        s_par = [0]

        def scores_exp(j, h, half):
            fp, g = h // 4, h % 4
            s_par[0] ^= 1
            S = psA.tile([128, 1024], F32, name=fresh(f"S{half}"),
                         tag=f"s{s_par[0]}")
            q3 = qk_q[fp][j // 4].rearrange("p (o t) -> p o t", o=2)
            qs = q3[g * 32:(g + 1) * 32, :,
                    (j % 4) * 128:(j % 4) * 128 + 128]
            for i in range(8):
                kc = half * 8 + i
                k3 = qk_k[fp][kc // 4].rearrange("p (o t) -> p o t", o=2)
                ks = k3[g * 32:(g + 1) * 32, :,
                        (kc % 4) * 128:(kc % 4) * 128 + 128]
                nc.tensor.matmul(S[:, i * 128:(i + 1) * 128], ks, qs,
                                 start=True, stop=True, perf_mode=DR,
                                 tile_position=(g * 32, 0))
            ex = expp.tile([128, 1024], BF16, name=fresh("ex"), tag="ex")
            nc.scalar.activation(ex, S, EXP, bias=0.0,
                                 scale=float(SCALE / (QK_SCALE * QK_SCALE)))
            return ex

        nrm_of = {}

        def norm_transpose(j, grp, avt):
            rcp = rcpp.tile([128, 4], F32, name=fresh("rcp"), tag="rcp")
            nc.vector.reciprocal(rcp, avt[:, 64::65])
            if grp == 0:
                nrm_of[j] = nrmp.tile([128, 512], BF16, name=fresh("nrm"),
                                      tag="nrm")
            nrm = nrm_of[j]
            for i in range(4):
                nc.vector.tensor_scalar_mul(
                    nrm[:, grp * 256 + i * 64:grp * 256 + (i + 1) * 64],
                    avt[:, i * 65:i * 65 + 64], rcp[:, i:i + 1])
            if grp == 1:
                # block-transpose [q, 4x128 feat] -> 4x [128 feat, q] in one
                # xbar DMA: out[p, m, f] = in[f, m*128 + p]
                aT_of[j] = aTp.tile([128, 512], BF16, name=fresh("aT"),
                                    tag="aT")
                nc.sync.dma_start_transpose(
                    aT_of[j].rearrange("p (m f) -> p m f", m=4),
                    nrm_of.pop(j))

        osb_of = {}

        def proj_unit(t, e):
            P = psF.tile([128, 512], F32, name=fresh("pp"), tag="f")
            aTt = aT_of[t] if e == 0 else aT_of.pop(t)
            for m in range(4):
                nc.tensor.matmul(P, aTt[:, m * 128:(m + 1) * 128],
                                 wproj_t[m][:, e * 512:(e + 1) * 512],
                                 start=(m == 0), stop=(m == 3))
            if e == 0:
                osb_of[t] = outp.tile([128, 1024], F32, name=fresh("osb"),
                                      tag="osb")
            osb = osb_of[t]
            nc.vector.tensor_add(osb[:, e * 512:(e + 1) * 512], P,
                                 bproj_t[:, e * 512:(e + 1) * 512])
            if e == 1:
                nc.sync.dma_start(out[t * 128:(t + 1) * 128, :],
                                  osb_of.pop(t))

        # ---------------- prologue ----------------
        # Critical path: k f'0 (all t-blocks) + q f'0 tb0 unlock the whole
        # first j-pair's hc0/hc1. First two units' bias-extracts ride the
        # still-idle ACT; later ones go to DVE so they don't delay exps.
        qkv2("k", 0, 0, True, borrow=psA.tile([128, 1024], F32,
                                              name=fresh("p0"), tag="s0"))
        qkv2("q", 0, 0, True, borrow=psA.tile([128, 1024], F32,
                                              name=fresh("p1"), tag="s1"))
        qkv2("k", 0, 1, False)
        qkv2("k", 0, 2, False)
        qkv2("k", 0, 3, False)
        load_xt_tb(0)
        load_xt_tb(1)
        # remaining prologue at natural priority, 1:1 interleaved: the f'1
        # k+q chunks gate jp0's hc2/hc3, the v units gate the (deferred)
        # jp0/jp1 AVs. v rotates through psV plus the AV banks, which are
        # provably idle until those deferred AVs run (same-tag handoff).
        qks = [("k", 1, 0), ("k", 1, 1), ("k", 1, 2), ("k", 1, 3),
               ("q", 1, 0), ("q", 0, 1), ("q", 1, 1)]
        vs = list(range(NKC))
        step = 0
        while qks or vs:
            if step == 4:
                load_xt_tb(2)
            if step == 8:
                load_xt_tb(3)
            if qks:
                qk, fp, tb = qks.pop(0)
                qkv2(qk, fp, tb, False)
            if vs:
                qkv_v(vs.pop(0), psV.tile([128, 512], F32,
                                          name=fresh("pv"), tag="v"))
            step += 1

        def emit_filler(u):
            with tc.high_priority(offset=-1_000_000):
                if u[0] == "qk":
                    qkv2("q", u[1], u[2], False)
                else:
                    proj_unit(u[1], u[2])

        # ---------------- attention loop ----------------
        # Deferred-AV pipeline: AV+norm work for j-pair n drains inline
        # (natural priority, in-order-safe) during j-pair n+2, by which
        # time the v units are done. The exp pool carries the 2-jp lag.
        def av(j, h, avt, ex0, ex1):
            cb = (h % 4) * 65
            for kc in range(NKC):
                ex = ex0 if kc < 8 else ex1
                nc.tensor.matmul(avt[:, cb:cb + 65],
                                 ex[:, (kc % 8) * 128:(kc % 8) * 128 + 128],
                                 vg[kc][:, h * 65:(h + 1) * 65],
                                 start=(kc == 0), stop=(kc == NKC - 1))

        av_q = []
        avt_of = {}

        def drain_av(k):
            while k > 0 and av_q:
                unit = av_q.pop(0)
                if unit[0] == "av":
                    _, j, h, ex0, ex1 = unit
                    grp = h // 4
                    if (j, grp) not in avt_of:
                        avt_of[(j, grp)] = psAV.tile(
                            [128, 260], F32, name=fresh(f"av{j}_{grp}"),
                            tag="av")
                    av(j, h, avt_of[(j, grp)], ex0, ex1)
                    k -= 1
                else:
                    _, j, grp = unit
                    norm_transpose(j, grp, avt_of.pop((j, grp)))

        fillers_for_j = {j: [] for j in range(NJ)}
        for tb in (2, 3):
            for fp in range(2):
                fillers_for_j[4 * (tb - 1) + 2 * fp - 2].append(("qk", fp, tb))
        # proj(t) needs norm(t), which drains 2 jps late early on and
        # catches up to ~1 jp by the end
        for t in range(NKC):
            jj = min(t + 5 if t < 8 else t + 3, NJ - 1)
            fillers_for_j[jj].append(("proj", t, 0))
            fillers_for_j[jj].append(("proj", t, 1))

        todo = []
        for jp in range(NJ // 2):
            todo += list(fillers_for_j.get(2 * jp - 1, ())) + \
                list(fillers_for_j.get(2 * jp, ()))
            if jp == 0:
                # match the k-chunk arrival order: f'0 tb0/1 exps first,
                # then f'0 tb2/3, then f'1 in the same half order
                exh = {}
                for fp in range(2):
                    for half in (0, 1):
                        for hc in (2 * fp, 2 * fp + 1):
                            for j in (0, 1):
                                for hp in range(2):
                                    h = 2 * hc + hp
                                    if half == 0:
                                        exh[(j, h)] = scores_exp(j, h, 0)
                                    else:
                                        ex1 = scores_exp(j, h, 1)
                                        av_q.append(("av", j, h,
                                                     exh.pop((j, h)), ex1))
                                        if h % 4 == 3 and j == 1 and hp == 1:
                                            pass
                            if half == 1 and hc % 2 == 1:
                                av_q.append(("norm", 0, fp))
                                av_q.append(("norm", 1, fp))
                continue
            for hc in range(4):
                grp = hc // 2
                for j in (2 * jp, 2 * jp + 1):
                    for hp in range(2):
                        h = 2 * hc + hp
                        ex0 = scores_exp(j, h, 0)
                        ex1 = scores_exp(j, h, 1)
                        av_q.append(("av", j, h, ex0, ex1))
                    if hc % 2 == 1:
                        av_q.append(("norm", j, grp))
                    if jp >= 3 or (jp == 2 and hc >= 2):
                        drain_av(4)
                if todo:
                    emit_filler(todo.pop(0))
                if todo:
                    emit_filler(todo.pop(0))
            todo += list(fillers_for_j.get(2 * jp + 1, ()))
            fillers_for_j[2 * jp + 1] = []
            todo += list(fillers_for_j.get(2 * jp + 2, ()))
            fillers_for_j[2 * jp + 2] = []
            if jp == NJ // 2 - 1:
                todo += list(fillers_for_j.get(NJ - 1, ()))
                while av_q or todo:
                    drain_av(2)
                    if todo:
                        emit_filler(todo.pop(0))


def _input_specs():
    import concourse.mybir as mybir
    BF16 = mybir.dt.bfloat16
    F32 = mybir.dt.float32
    return {
        "xT": ((C, T), BF16), "xT8": ((128, 8 * T), mybir.dt.float8e4),
        "wqk8": ((128, 8 * C), mybir.dt.float8e4), "wv": ((C, 512), BF16),
        "wproj": ((C // G, C), BF16),
        "bqk": ((128, 8), F32), "bv": ((128, 520), BF16),
        "bproj": ((128, 1024), BF16),
        "csT": ((128, 2 * T), BF16),
    }


def _build_program():
    import concourse.mybir as mybir
    import concourse.tile as tile
    from concourse import bacc

    nc = bacc.Bacc("TRN2", target_bir_lowering=False, debug=False)
    ins = {}
    for name, (shape, dt) in _input_specs().items():
        ins[name] = nc.dram_tensor(name, list(shape), dt,
                                   kind="ExternalInput").ap()
    outs = {"out": nc.dram_tensor("out", [T, C], mybir.dt.float32,
                                  kind="ExternalOutput").ap()}
    with tile.TileContext(nc) as tc:
        _attn_body(tc, outs, ins)
    nc.compile()
    return nc


def _core_inputs(core, x, W_qkv, b_qkv, W_proj, b_proj, cosT, sinT):
    import ml_dtypes
    bf16 = ml_dtypes.bfloat16
    fp8 = ml_dtypes.float8_e4m3fn
    f32 = np.float32
    b, g = divmod(core, 2)
    xTa = np.ascontiguousarray(np.asarray(x[b], dtype=f32).T).astype(bf16)
    W_qkv = np.asarray(W_qkv, dtype=f32)
    b_qkv = np.asarray(b_qkv, dtype=f32)

    # column selection for the packed (dd-pair) layout: block bc covers
    # features (head 4*fp + gg, d = dd*32 + p) at rows 32*gg + p
    def col_of(qk, fp, dd, gg, p):
        base = 0 if qk == "q" else C
        return base + g * 512 + (4 * fp + gg) * 64 + dd * 32 + p

    wqk_p = np.zeros((C, 1024), f32)
    bqkt = np.zeros((128, 8), f32)
    for (qk, fp, dd), bc in BC.items():
        for gg in range(4):
            for p in range(32):
                c = col_of(qk, fp, dd, gg, p)
                wqk_p[:, bc * 128 + gg * 32 + p] = W_qkv[:, c]
                bqkt[gg * 32 + p, bc] = b_qkv[c]

    # fp8 DoubleRow inputs for the q/k projection: row p of pair-block P
    # holds c-chunks (2P, 2P+1) as the two DR k-tiles (16x weight scaling
    # keeps W out of the fp8 subnormal range; undone in raw extraction)
    wqk8 = np.zeros((128, 8 * C), np.float32)
    xq = np.asarray(x[b], dtype=f32)          # (T, C)
    xT8 = np.zeros((128, 8 * T), np.float32)
    for P in range(4):
        for o in range(2):
            ch = (2 * P + o) * 128
            wqk8[:, (2 * P + o) * 1024:(2 * P + o + 1) * 1024] = \
                16.0 * wqk_p[ch:ch + 128, :]
            xT8[:, (2 * P + o) * T:(2 * P + o + 1) * T] = \
                xq[:, ch:ch + 128].T
    wqk8 = np.ascontiguousarray(wqk8).astype(fp8)
    xT8 = np.ascontiguousarray(xT8).astype(fp8)

    v = W_qkv[:, 2 * C + g * 512:2 * C + (g + 1) * 512]
    wva = np.ascontiguousarray(v).astype(bf16)
    bvr = b_qkv[2 * C + g * 512:2 * C + (g + 1) * 512].reshape(8, 64)
    bvg = np.concatenate([bvr, np.ones((8, 1), f32)], axis=1).reshape(-1)
    bva = np.ascontiguousarray(np.tile(bvg[None, :], (128, 1))).astype(bf16)
    wpa = np.ascontiguousarray(
        np.asarray(W_proj, dtype=f32)[g * 512:(g + 1) * 512]).astype(bf16)
    if g == 0:
        bpa = np.ascontiguousarray(
            np.tile(np.asarray(b_proj, dtype=f32)[None, :], (128, 1)))
    else:
        bpa = np.zeros((128, C), dtype=f32)
    bpa = bpa.astype(bf16)
    csT = np.ascontiguousarray(
        np.concatenate([cosT, sinT], axis=1)).astype(bf16)
    return {"xT": xTa, "xT8": xT8, "wqk8": wqk8, "wv": wva, "wproj": wpa,
            "bqk": bqkt, "bv": bva, "bproj": bpa, "csT": csT}


def run(x, W_qkv, b_qkv, W_proj, b_proj, trace=False):
    from concourse.bass_utils import run_bass_kernel_spmd

    if "nc" not in _CACHED:
        _CACHED["nc"] = _build_program()
    nc = _CACHED["nc"]

    cosT, sinT = _rope_tables()
    in_maps = [_core_inputs(c, x, W_qkv, b_qkv, W_proj, b_proj, cosT, sinT)
               for c in range(8)]
    res = run_bass_kernel_spmd(nc, in_maps, core_ids=list(range(8)), trace=trace)
    parts = [r["out"] for r in res.results]
    outv = np.stack([parts[2 * b] + parts[2 * b + 1] for b in range(B)], axis=0)
    return outv.astype(np.float32), res


def kernel(x, W_qkv, b_qkv, W_proj, b_proj):
    outv, _ = run(x, W_qkv, b_qkv, W_proj, b_proj, trace=False)
    return outv
